# revision 6
# baseline (speedup 1.0000x reference)
"""Chamfer distance (nn_ChamferLoss) Trainium2 kernel.

Inputs: x [32, 2048, 3] f32, y [32, 2048, 3] f32.
Output: scalar f32 = mean_b( mean_n min_m d + mean_m min_n d ),
        d[b,i,j] = ||x[b,i] - y[b,j]||^2.

Strategy (8 NeuronCores, pure data parallel over batch, 4 batches/core):
- d = ||x||^2 + ||y||^2 - 2 x.y.  Per batch and direction, compute
  A[j,i] = aug(x)_j . aug(y)_i = ||y_i||^2 - 2 x_j.y_i on the PE as a
  K=4 augmented matmul; then min_i A + ||x_j||^2 gives the row mins.
- fp32 exactness at bf16 speed: each fp32 operand is split into three
  bf16 limbs (h+m+l); the 6 significant limb-product pairs are stacked
  into the contraction dim (K = 6*4 = 24).  bf16 streams at 1 col/cycle
  vs 4 for native fp32 matmul; products are exact in the PE, accumulated
  in fp32 PSUM.
- The 2048x2048 distance matrix per (batch, direction) is produced in
  [128, 2048] PSUM chunks (16 x-chunks).  The scalar engine copies half
  of each chunk PSUM->SBUF; a custom DVE op (min(in0,in1) elementwise
  with fused min-reduction and seedable accumulator) consumes the other
  PSUM half and the SBUF half in one pass (2 elements/cycle).
- Per-chunk row-mins land in a [128, 128] result tile, DMA'd out; the
  tiny final sums/means (plus the ||.||^2 offsets) happen on host in
  fp64, and the 8 per-core partials are averaged on host.
"""
import sys

for _p in ("/opt/trn_rl_repo", "/root/.axon_site/_ro/trn_rl_repo"):
    if _p not in sys.path:
        sys.path.append(_p)

import numpy as np
import ml_dtypes

import concourse.bacc as bacc
import concourse.tile as tile
import concourse.mybir as mybir
from concourse import bass_utils
import concourse.dve_ops as dve_ops
from concourse.dve_ops import DveOp
from concourse.dve_spec import Spec, Src0, Src1, C0, minn, lower
from concourse.dve_uop import DveOpSpec

B, N, M = 32, 2048, 2048
NCORES = 8
BPC = B // NCORES          # batches per core
NBO = BPC * 2              # (batch, direction) pairs per core
NCHUNK = N // 128          # x-chunks per pair
KAUG = 4                   # augmented coordinate count
KSPLIT = 6                 # bf16 limb-product pairs kept
K = KAUG * KSPLIT          # matmul contraction dim
SEED = 3.0e38

_BF16 = ml_dtypes.bfloat16


# --------------------------------------------------------------------------
# custom DVE op: out = min(in0, in1); accum_out = min(min_k out[k], s0)
# --------------------------------------------------------------------------
def _ttmr_ref(in0, in1, c0, c1, c2):
    body = np.minimum(in0.astype(np.float32), in1.astype(np.float32))
    acc = np.minimum(
        body.reshape(body.shape[0], -1).min(axis=-1),
        np.asarray(c0, np.float32).reshape(-1),
    )
    return body, acc


def _register_min_reduce_op() -> DveOp:
    name = "TENSOR_TENSOR_MIN_REDUCE_ANT"
    for op in dve_ops.OPS:
        if op.name == name:
            return op
    spec = Spec(body=minn(Src0, Src1), accum=minn, accum_init=C0, reference=_ttmr_ref)
    op = DveOp(name, spec, subdim=False, uops_sha={})
    dve_ops.OPS.append(op)
    dve_ops.CUSTOM_DVE_SPECS[name] = spec
    row = dve_ops._CUSTOM_DVE_ROW_BASE + len(dve_ops.OPS) - 1
    assert row < 0x20
    dve_ops._SUB_OPCODE_FOR_NAME[name] = row
    shas = {}
    for ver in ("v3", "v4"):
        shas[ver] = DveOpSpec(
            name=name, opcode=row, uops=lower(spec, ver=ver), rd1_en=True
        ).sha(ver)
    object.__setattr__(op, "uops_sha", shas)
    return op


# --------------------------------------------------------------------------
# device kernel build
# --------------------------------------------------------------------------
_NC_CACHE: dict = {}


def _build_nc(reps: int = 1, loop: int = 1):
    key = (reps, loop)
    if key in _NC_CACHE:
        return _NC_CACHE[key]
    ttmr = _register_min_reduce_op()
    nc = bacc.Bacc("TRN2", target_bir_lowering=False, debug=False)
    lhs_d = nc.dram_tensor("lhs", [NBO, K, N], mybir.dt.bfloat16, kind="ExternalInput")
    rhs_d = nc.dram_tensor("rhs", [NBO, K, M], mybir.dt.bfloat16, kind="ExternalInput")
    out_d = nc.dram_tensor(
        "out", [128, NBO * NCHUNK], mybir.dt.float32, kind="ExternalOutput"
    )

    half = M // 2
    with tile.TileContext(nc) as tc:
        with (
            tc.tile_pool(name="lp", bufs=2) as lp,
            tc.tile_pool(name="rp", bufs=2) as rp,
            tc.tile_pool(name="cp", bufs=6) as cp,
            tc.tile_pool(name="tp", bufs=4) as tp,
            tc.tile_pool(name="res", bufs=NBO) as resp,
            tc.tile_pool(name="ps1", bufs=2, space="PSUM") as ps1,
            tc.tile_pool(name="ps2", bufs=2, space="PSUM") as ps2,
        ):
            # one result tile per (batch, direction): a single shared tile
            # adds cross-op dependency bookkeeping that measurably slows DVE
            res_tiles = [
                resp.tile([128, NCHUNK], mybir.dt.float32,
                          name=f"res{i}", tag=f"res{i}")
                for i in range(NBO)
            ]

            def body():
                for bo in range(NBO):
                    lt = lp.tile([K, N], mybir.dt.bfloat16)
                    nc.sync.dma_start(lt[:], lhs_d[bo])
                    rt = rp.tile([K, M], mybir.dt.bfloat16)
                    nc.sync.dma_start(rt[:], rhs_d[bo])
                    for c in range(NCHUNK):
                        cu = bo * NCHUNK + c
                        lts = lt[:, c * 128:(c + 1) * 128]
                        # two independent PSUM tiles per chunk: pA feeds the
                        # scalar-engine copy, pB feeds the DVE directly, so
                        # deps/releases don't serialize the pipeline
                        pA = ps1.tile([128, half], mybir.dt.float32)
                        pB = ps2.tile([128, half], mybir.dt.float32)
                        for k in range(half // 512):
                            nc.tensor.matmul(
                                pA[:, k * 512:(k + 1) * 512],
                                lts,
                                rt[:, k * 512:(k + 1) * 512],
                                start=True,
                                stop=True,
                            )
                        for k in range(half // 512):
                            nc.tensor.matmul(
                                pB[:, k * 512:(k + 1) * 512],
                                lts,
                                rt[:, half + k * 512:half + (k + 1) * 512],
                                start=True,
                                stop=True,
                            )
                        cpy = cp.tile([128, half], mybir.dt.float32)
                        nc.scalar.copy(cpy[:], pA[:])
                        trash = tp.tile([128, half], mybir.dt.float32)
                        nc.vector._custom_dve(
                            ttmr,
                            out=trash[:],
                            in0=pB[:],
                            in1=cpy[:],
                            s0=SEED,
                            accum_out=res_tiles[bo][:, c:c + 1],
                        )

            if loop == 1:
                for _ in range(reps):
                    body()
            else:
                with tc.For_i(0, loop, 1):
                    for _ in range(reps):
                        body()
            for bo in range(NBO):
                nc.sync.dma_start(
                    out_d[:, bo * NCHUNK:(bo + 1) * NCHUNK], res_tiles[bo][:]
                )

    nc.compile()
    _NC_CACHE[key] = nc
    return nc


# --------------------------------------------------------------------------
# host side
# --------------------------------------------------------------------------
def _split3(v: np.ndarray):
    """v (f32) -> three bf16 limbs with v ~= h + m + l exactly to ~2^-26."""
    h = v.astype(_BF16)
    r = v - h.astype(np.float32)
    m = r.astype(_BF16)
    l = (r - m.astype(np.float32)).astype(_BF16)
    return h, m, l


def _build_inputs(x: np.ndarray, y: np.ndarray):
    """Build per-core in_maps plus the host-side norm sums."""
    x = np.ascontiguousarray(x, dtype=np.float32)
    y = np.ascontiguousarray(y, dtype=np.float32)
    xt = x.transpose(0, 2, 1)  # [B, 3, N]
    yt = y.transpose(0, 2, 1)
    xn = (x.astype(np.float64) ** 2).sum(-1)  # [B, N]
    yn = (y.astype(np.float64) ** 2).sum(-1)

    # stationary side a, moving side b; direction 0: rows of x vs all y,
    # direction 1: rows of y vs all x.
    A = np.empty((B, 2, KAUG, N), np.float32)
    Bm = np.empty((B, 2, KAUG, M), np.float32)
    A[:, 0, :3] = xt
    A[:, 0, 3] = 1.0
    A[:, 1, :3] = yt
    A[:, 1, 3] = 1.0
    Bm[:, 0, :3] = -2.0 * yt
    Bm[:, 0, 3] = yn.astype(np.float32)
    Bm[:, 1, :3] = -2.0 * xt
    Bm[:, 1, 3] = xn.astype(np.float32)

    ah, am, al = _split3(A)
    bh, bm, bl = _split3(Bm)
    # kept limb products: hh, mh, lh, hm, mm, hl
    LHS = np.concatenate([ah, am, al, ah, am, ah], axis=2)  # [B, 2, 24, N]
    RHS = np.concatenate([bh, bh, bh, bm, bm, bl], axis=2)

    LHS = LHS.reshape(NCORES, NBO, K, N)
    RHS = RHS.reshape(NCORES, NBO, K, M)
    in_maps = [
        {"lhs": np.ascontiguousarray(LHS[c]), "rhs": np.ascontiguousarray(RHS[c])}
        for c in range(NCORES)
    ]
    return in_maps, xn, yn


def _finalize(results, xn, yn) -> np.ndarray:
    per_item = np.empty(B, np.float64)
    for core in range(NCORES):
        res = results[core]["out"].astype(np.float64)  # [128, NBO*16]
        for lb in range(BPC):
            b = core * BPC + lb
            s0 = res[:, (lb * 2) * NCHUNK:(lb * 2 + 1) * NCHUNK].sum()
            s1 = res[:, (lb * 2 + 1) * NCHUNK:(lb * 2 + 2) * NCHUNK].sum()
            x_min_sum = s0 + xn[b].sum()
            y_min_sum = s1 + yn[b].sum()
            per_item[b] = x_min_sum / N + y_min_sum / M
    return np.asarray(per_item.mean(), dtype=np.float32)


def _run(x: np.ndarray, y: np.ndarray, reps: int = 1):
    nc = _build_nc(reps)
    in_maps, xn, yn = _build_inputs(x, y)
    res = bass_utils.run_bass_kernel_spmd(nc, in_maps, core_ids=list(range(NCORES)))
    return _finalize(res.results, xn, yn)


def kernel(x: np.ndarray, y: np.ndarray) -> np.ndarray:
    return _run(x, y, reps=1)
